# revision 9
# baseline (speedup 1.0000x reference)
"""DN4/MetaBaseline few-shot head on 8 Trainium2 cores.

Problem shapes (hardcoded): x_shot [4,5,5,640,10,10], x_query [4,50,640,10,10].
Sharding: core i handles episode b = i//2 and query half h = i%2 (25 queries).
Outputs: logits_cos [4,50,5], logits_dn4 [4,50,5], plus r_cos/r_dn4 passthrough.

Per-core pipeline:
  - support/query descriptor L2 norms via bf16 squares + ones-column matmul
    (partition-dim sum) + DVE reciprocal (+ sqrt),
  - support descriptors scaled by 1/||s|| and rounded to bf16,
  - sims = q_desc^T s_desc as 625 bf16 matmuls [K=128, M=100, N=500] into PSUM,
  - top-5 per row via the DVE max (top-8) instruction straight from PSUM,
  - sum of squared top-5 via ACT Square with accum_out,
  - 1/||q||^2 row scaling folded into a final K=100 matmul against a
    transposed-reciprocal matrix, then sqrt,
  - cosine logits from pooled sums (scale-invariant) + tiny matmuls.
"""

from contextlib import ExitStack

import numpy as np

import concourse.bacc as bacc
import concourse.tile as tile
from concourse import mybir
from concourse.alu_op_type import AluOpType

F32 = mybir.dt.float32
BF16 = mybir.dt.bfloat16
AF = mybir.ActivationFunctionType
AX = mybir.AxisListType

B, WAY, SHOT, C, HWDIM = 4, 5, 5, 640, 100
NQ = 25          # queries per core
NCH = 5          # C chunks of 128
CP = 128
Y = SHOT * HWDIM  # 500 support descriptors per way
KNN = 5
DN4_SCALE = 1.0 / float((KNN * 50) ** 2)  # sqrt(x * scale) == sqrt(x) / (k * hw_q)

MAX_FROM_PSUM = True


def build_program():
    nc = bacc.Bacc("TRN2", target_bir_lowering=False, debug=False)
    xs = nc.dram_tensor("xs", [NCH, CP, WAY * Y], F32, kind="ExternalInput")
    xq = nc.dram_tensor("xq", [NCH, CP, NQ * HWDIM], F32, kind="ExternalInput")
    out_cos = nc.dram_tensor("out_cos", [NQ, WAY], F32, kind="ExternalOutput")
    out_dn4 = nc.dram_tensor("out_dn4", [NQ * WAY], F32, kind="ExternalOutput")

    with tile.TileContext(nc) as tc:
        with ExitStack() as ctx:
            sb_const = ctx.enter_context(tc.tile_pool(name="sb_const", bufs=1))
            sb_bf = ctx.enter_context(tc.tile_pool(name="sb_bf", bufs=NCH))
            sb_one = ctx.enter_context(tc.tile_pool(name="sb_one", bufs=1))
            sb_loop = ctx.enter_context(tc.tile_pool(name="sb_loop", bufs=4))

            ones_row = sb_const.tile([1, CP], F32)
            nc.vector.memset(ones_row[:], 1.0)
            ones_col = sb_const.tile([CP, 1], BF16)
            nc.vector.memset(ones_col[:], 1.0)

            # persistent tiles
            s_bf = [
                sb_bf.tile([CP, WAY * Y], BF16, tag="s_bf", name=f"s_bf{i}")
                for i in range(NCH)
            ]
            q_bf = [
                sb_bf.tile([CP, NQ * HWDIM], BF16, tag="q_bf", name=f"q_bf{i}")
                for i in range(NCH)
            ]
            smean = sb_one.tile([CP, NCH * WAY * SHOT], F32, tag="smean")
            proto_t = sb_one.tile([CP, NCH * WAY], F32, tag="proto")
            qm_t = sb_one.tile([CP, NCH * NQ], F32, tag="qm")
            pinv = sb_one.tile([1, WAY], F32, tag="pinv")
            pinv_b = sb_one.tile([NQ, WAY], F32, tag="pinv_b")
            qminv = sb_one.tile([1, NQ], F32, tag="qminv")
            qminv_col = sb_one.tile([NQ, 1], F32, tag="qminv_col")
            qmat = sb_one.tile([HWDIM, NQ], F32, tag="qmat")
            dn4row = sb_one.tile([1, NQ * WAY], F32, tag="dn4row")
            cos_sb = sb_one.tile([NQ, WAY], F32, tag="cos_sb")

            # ---------------- prep phase (own SBUF/PSUM scope) ----------------
            with ExitStack() as pctx:
                sb_data = pctx.enter_context(tc.tile_pool(name="sb_data", bufs=NCH))
                sb_scratch = pctx.enter_context(tc.tile_pool(name="sb_scr", bufs=2))
                sb_prep = pctx.enter_context(tc.tile_pool(name="sb_prep", bufs=1))
                ps_acc = pctx.enter_context(
                    tc.tile_pool(name="ps_acc", bufs=1, space="PSUM")
                )
                ps_bcast = pctx.enter_context(
                    tc.tile_pool(name="ps_bcast", bufs=2, space="PSUM")
                )
                ps_qmat = pctx.enter_context(
                    tc.tile_pool(name="ps_qmat", bufs=1, space="PSUM")
                )

                # ---- load inputs ----
                s_raw = []
                q_raw = []
                for ci in range(NCH):
                    st = sb_data.tile([CP, WAY * Y], F32, tag="s")
                    nc.sync.dma_start(st[:], xs[ci])
                    s_raw.append(st)
                for ci in range(NCH):
                    qt = sb_data.tile([CP, NQ * HWDIM], F32, tag="q")
                    nc.sync.dma_start(qt[:], xq[ci])
                    q_raw.append(qt)

                sinv_row = sb_prep.tile([1, WAY * Y], F32, tag="sinv_row")
                qinv2_row = sb_prep.tile([1, NQ * HWDIM], F32, tag="qinv2_row")
                sinv_b = sb_prep.tile([CP, WAY * Y], F32, tag="sinv_b")

                # -- support: pooled sums for the cosine path (raw features;
                #    cosine is invariant to the positive mean scaling) --
                for ci in range(NCH):
                    nc.vector.reduce_sum(
                        smean[:, ci * 25 : (ci + 1) * 25],
                        s_raw[ci][:].rearrange("p (n k) -> p n k", k=HWDIM),
                        AX.X,
                    )
                for ci in range(NCH):
                    nc.vector.reduce_sum(
                        proto_t[:, ci * WAY : (ci + 1) * WAY],
                        smean[:, ci * 25 : (ci + 1) * 25].rearrange(
                            "p (n k) -> p n k", k=SHOT
                        ),
                        AX.X,
                    )

                # -- support: per-descriptor L2 norms -> sinv broadcast --
                ssq = ps_acc.tile([1, 5, 512], F32, tag="acc")
                for ci in range(NCH):
                    sqt = sb_scratch.tile([CP, WAY * Y], BF16, tag="sq")
                    nc.scalar.square(sqt[:], s_raw[ci][:])
                    for j in range(5):
                        nc.tensor.matmul(
                            ssq[0:1, j, 0:500],
                            ones_col[:],
                            sqt[:, j * 500 : (j + 1) * 500],
                            start=(ci == 0),
                            stop=(ci == NCH - 1),
                        )
                for j in range(5):
                    nc.vector.reciprocal(
                        sinv_row[:, j * 500 : (j + 1) * 500], ssq[0:1, j, 0:500]
                    )
                    nc.scalar.activation(
                        sinv_row[:, j * 500 : (j + 1) * 500],
                        sinv_row[:, j * 500 : (j + 1) * 500],
                        AF.Sqrt,
                    )
                for j in range(5):
                    bc = ps_bcast.tile([CP, 500], F32, tag="bc")
                    nc.tensor.matmul(
                        bc[:],
                        ones_row[0:1, :],
                        sinv_row[:, j * 500 : (j + 1) * 500],
                        start=True,
                        stop=True,
                    )
                    nc.scalar.copy(sinv_b[:, j * 500 : (j + 1) * 500], bc[:])
                # scale support descriptors, rounding to bf16
                for ci in range(NCH):
                    nc.vector.tensor_mul(s_bf[ci][:], s_raw[ci][:], sinv_b[:])
                # round query descriptors to bf16 (gpsimd is otherwise idle)
                for ci in range(NCH):
                    nc.gpsimd.tensor_copy(q_bf[ci][:], q_raw[ci][:])

                # -- support: prototype norms (cosine path) --
                sqp = sb_scratch.tile([CP, WAY * Y], BF16, tag="sq")
                nc.scalar.square(sqp[:, 0 : NCH * WAY], proto_t[:])
                psq = ps_acc.tile([1, 5, 512], F32, tag="acc")
                for ci in range(NCH):
                    nc.tensor.matmul(
                        psq[0:1, 0, 0:WAY],
                        ones_col[:],
                        sqp[:, ci * WAY : (ci + 1) * WAY],
                        start=(ci == 0),
                        stop=(ci == NCH - 1),
                    )
                nc.vector.reciprocal(pinv[:], psq[0:1, 0, 0:WAY])
                nc.scalar.activation(pinv[:], pinv[:], AF.Sqrt)
                bcp = ps_bcast.tile([CP, 500], F32, tag="bc")
                nc.tensor.matmul(
                    bcp[0:NQ, 0:WAY],
                    ones_row[0:1, 0:NQ],
                    pinv[:],
                    start=True,
                    stop=True,
                )
                nc.scalar.copy(pinv_b[:], bcp[0:NQ, 0:WAY])

                # -- query: pooled sums + norms (cosine path) --
                for ci in range(NCH):
                    nc.vector.reduce_sum(
                        qm_t[:, ci * NQ : (ci + 1) * NQ],
                        q_raw[ci][:].rearrange("p (n k) -> p n k", k=HWDIM),
                        AX.X,
                    )
                sqm = sb_scratch.tile([CP, WAY * Y], BF16, tag="sq")
                nc.scalar.square(sqm[:, 0 : NCH * NQ], qm_t[:])
                qmsq = ps_acc.tile([1, 5, 512], F32, tag="acc")
                for ci in range(NCH):
                    nc.tensor.matmul(
                        qmsq[0:1, 0, 0:NQ],
                        ones_col[:],
                        sqm[:, ci * NQ : (ci + 1) * NQ],
                        start=(ci == 0),
                        stop=(ci == NCH - 1),
                    )
                nc.vector.reciprocal(qminv[:], qmsq[0:1, 0, 0:NQ])
                nc.scalar.activation(qminv[:], qminv[:], AF.Sqrt)
                qmc = ps_qmat.tile([HWDIM, NQ], F32, tag="qmat")
                nc.tensor.matmul(
                    qmc[0:NQ, 0:1], qminv[:], ones_row[0:1, 0:1], start=True, stop=True
                )
                nc.scalar.copy(qminv_col[:], qmc[0:NQ, 0:1])

                # -- query: per-descriptor squared norms -> 1/||q||^2 matrix --
                qsq = ps_acc.tile([1, 5, 512], F32, tag="acc")
                for ci in range(NCH):
                    sqq = sb_scratch.tile([CP, WAY * Y], BF16, tag="sq")
                    nc.scalar.square(sqq[:, 0 : NQ * HWDIM], q_raw[ci][:])
                    for j in range(5):
                        nc.tensor.matmul(
                            qsq[0:1, j, 0:500],
                            ones_col[:],
                            sqq[:, j * 500 : (j + 1) * 500],
                            start=(ci == 0),
                            stop=(ci == NCH - 1),
                        )
                for j in range(5):
                    nc.vector.reciprocal(
                        qinv2_row[:, j * 500 : (j + 1) * 500], qsq[0:1, j, 0:500]
                    )
                qmp = ps_qmat.tile([HWDIM, NQ], F32, tag="qmat")
                for q in range(NQ):
                    nc.tensor.matmul(
                        qmp[:, q : q + 1],
                        qinv2_row[:, q * HWDIM : (q + 1) * HWDIM],
                        ones_row[0:1, 0:1],
                        start=True,
                        stop=True,
                    )
                nc.scalar.copy(qmat[:], qmp[:])

            # ---------------- main phase ----------------
            with ExitStack() as mctx:
                ps_sims = mctx.enter_context(
                    tc.tile_pool(name="ps_sims", bufs=5, space="PSUM")
                )
                ps_dn4 = mctx.enter_context(
                    tc.tile_pool(name="ps_dn4", bufs=2, space="PSUM")
                )
                ps_dots = mctx.enter_context(
                    tc.tile_pool(name="ps_dots", bufs=1, space="PSUM")
                )

                # cosine logits
                dots = ps_dots.tile([NQ, WAY], F32, tag="dots")
                for ci in range(NCH):
                    nc.tensor.matmul(
                        dots[:],
                        qm_t[:, ci * NQ : (ci + 1) * NQ],
                        proto_t[:, ci * WAY : (ci + 1) * WAY],
                        start=(ci == 0),
                        stop=(ci == NCH - 1),
                    )
                nc.vector.scalar_tensor_tensor(
                    cos_sb[:],
                    dots[:],
                    qminv_col[:],
                    pinv_b[:],
                    AluOpType.mult,
                    AluOpType.mult,
                )
                nc.sync.dma_start(out_cos[:, :], cos_sb[:])

                # DN4 logits
                for q in range(NQ):
                    t_all = sb_loop.tile([HWDIM, WAY], F32, tag="t_all")
                    for w in range(WAY):
                        ps = ps_sims.tile([HWDIM, Y], F32, tag="sims")
                        for ci in range(NCH):
                            nc.tensor.matmul(
                                ps[:],
                                q_bf[ci][:, q * HWDIM : (q + 1) * HWDIM],
                                s_bf[ci][:, w * Y : (w + 1) * Y],
                                start=(ci == 0),
                                stop=(ci == NCH - 1),
                            )
                        m8 = sb_loop.tile([HWDIM, 8], F32, tag="m8")
                        if MAX_FROM_PSUM:
                            nc.vector.max(m8[:], ps[:])
                        else:
                            sco = sb_loop.tile([HWDIM, Y], F32, tag="sco")
                            nc.scalar.copy(sco[:], ps[:])
                            nc.vector.max(m8[:], sco[:])
                        scr5 = sb_loop.tile([HWDIM, KNN], F32, tag="scr5")
                        nc.scalar.activation(
                            scr5[:],
                            m8[:, 0:KNN],
                            AF.Square,
                            accum_out=t_all[:, w : w + 1],
                        )
                    dps = ps_dn4.tile([1, WAY], F32, tag="dn4")
                    nc.tensor.matmul(
                        dps[:],
                        qmat[:, q : q + 1],
                        t_all[:],
                        start=True,
                        stop=True,
                    )
                    nc.scalar.activation(
                        dn4row[:, q * WAY : (q + 1) * WAY],
                        dps[:],
                        AF.Sqrt,
                        scale=DN4_SCALE,
                    )
                nc.sync.dma_start(out_dn4[:], dn4row[:])

    nc.finalize()
    return nc


_NC_CACHE = None


def _get_program():
    global _NC_CACHE
    if _NC_CACHE is None:
        _NC_CACHE = build_program()
    return _NC_CACHE


def make_in_maps(x_shot, x_query):
    x_shot = np.asarray(x_shot, dtype=np.float32)
    x_query = np.asarray(x_query, dtype=np.float32)
    in_maps = []
    for core in range(8):
        b, h = divmod(core, 2)
        xs_r = (
            x_shot[b]
            .reshape(WAY * SHOT, NCH, CP, HWDIM)
            .transpose(1, 2, 0, 3)
            .reshape(NCH, CP, WAY * Y)
        )
        xq_r = (
            x_query[b, h * NQ : (h + 1) * NQ]
            .reshape(NQ, NCH, CP, HWDIM)
            .transpose(1, 2, 0, 3)
            .reshape(NCH, CP, NQ * HWDIM)
        )
        in_maps.append(
            {
                "xs": np.ascontiguousarray(xs_r),
                "xq": np.ascontiguousarray(xq_r),
            }
        )
    return in_maps


def assemble(results):
    cos = np.zeros((B, 50, WAY), np.float32)
    dn4 = np.zeros((B, 50, WAY), np.float32)
    for core in range(8):
        b, h = divmod(core, 2)
        cos[b, h * NQ : (h + 1) * NQ] = results[core]["out_cos"]
        dn4[b, h * NQ : (h + 1) * NQ] = results[core]["out_dn4"].reshape(NQ, WAY)
    return cos, dn4


def kernel(x_shot, x_query, r_cos, r_dn4, neighbor_k=5):
    from concourse.bass_utils import run_bass_kernel_spmd

    nc = _get_program()
    in_maps = make_in_maps(x_shot, x_query)
    r = run_bass_kernel_spmd(nc, in_maps, list(range(8)))
    cos, dn4 = assemble(r.results)
    return (
        cos,
        dn4,
        np.asarray(r_cos, np.float32),
        np.asarray(r_dn4, np.float32),
    )


# revision 23
# speedup vs baseline: 1.2037x; 1.2037x over previous
"""DN4/MetaBaseline few-shot head on 8 Trainium2 cores.

Problem shapes (hardcoded): x_shot [4,5,5,640,10,10], x_query [4,50,640,10,10].
Sharding: core i handles episode b = i//2 and query half h = i%2 (25 queries).
Outputs: logits_cos [4,50,5], logits_dn4 [4,50,5], plus r_cos/r_dn4 passthrough.

Per-core pipeline:
  - support/query descriptor L2 norms via bf16 squares + ones-column matmul
    (partition-dim sum) + DVE reciprocal (+ sqrt),
  - support descriptors scaled by 1/||s|| and rounded to bf16,
  - sims = q_desc^T s_desc as 625 bf16 matmuls [K=128, M=100, N=500] into PSUM,
  - top-5 per row via the DVE max (top-8) instruction straight from PSUM,
  - sum of squared top-5 via ACT Square with accum_out,
  - 1/||q||^2 row scaling folded into a final K=100 matmul against a
    transposed-reciprocal matrix, then sqrt,
  - cosine logits from pooled sums (scale-invariant) + tiny matmuls.
"""

from contextlib import ExitStack

import numpy as np

import concourse.bacc as bacc
import concourse.tile as tile
from concourse import mybir
from concourse.alu_op_type import AluOpType

F32 = mybir.dt.float32
BF16 = mybir.dt.bfloat16
AF = mybir.ActivationFunctionType
AX = mybir.AxisListType

B, WAY, SHOT, C, HWDIM = 4, 5, 5, 640, 100
NQ = 25          # queries per core
NCH = 5          # C chunks of 128
CP = 128
Y = SHOT * HWDIM  # 500 support descriptors per way
KNN = 5
DN4_SCALE = 1.0 / float((KNN * 50) ** 2)  # sqrt(x * scale) == sqrt(x) / (k * hw_q)

MAX_FROM_PSUM = True


NM = 20  # ceil(NQ*HWDIM / 128) M-chunks of packed query descriptors


def _rows(m):
    return min(CP, NQ * HWDIM - m * CP)


def build_program():
    nc = bacc.Bacc("TRN2", target_bir_lowering=False, debug=False)
    xs = nc.dram_tensor("xs", [NCH, CP, WAY * Y], BF16, kind="ExternalInput")
    xq = nc.dram_tensor("xq", [NCH, CP, NQ * HWDIM], BF16, kind="ExternalInput")
    seg = nc.dram_tensor("seg", [CP, NM * NQ], F32, kind="ExternalInput")
    out_cos = nc.dram_tensor("out_cos", [NQ, WAY], F32, kind="ExternalOutput")
    out_dn4 = nc.dram_tensor("out_dn4", [NQ * WAY], F32, kind="ExternalOutput")

    with tile.TileContext(nc) as tc:
        with ExitStack() as ctx:
            sb_const = ctx.enter_context(tc.tile_pool(name="sb_const", bufs=1))
            sb_bf = ctx.enter_context(tc.tile_pool(name="sb_bf", bufs=NCH))
            sb_one = ctx.enter_context(tc.tile_pool(name="sb_one", bufs=1))
            sb_loop = ctx.enter_context(tc.tile_pool(name="sb_loop", bufs=4))

            ones_row = sb_const.tile([1, CP], F32)
            nc.vector.memset(ones_row[:], 1.0)
            ones_col = sb_const.tile([CP, 1], BF16)
            nc.vector.memset(ones_col[:], 1.0)

            # persistent tiles
            s_bf = [
                sb_bf.tile([CP, WAY * Y], BF16, tag="s_bf", name=f"s_bf{i}")
                for i in range(NCH)
            ]
            q_bf = [
                sb_bf.tile([CP, NQ * HWDIM], BF16, tag="q_bf", name=f"q_bf{i}")
                for i in range(NCH)
            ]
            smean = sb_one.tile([CP, NCH * WAY * SHOT], F32, tag="smean")
            proto_t = sb_one.tile([CP, NCH * WAY], F32, tag="proto")
            qm_t = sb_one.tile([CP, NCH * NQ], F32, tag="qm")
            pinv = sb_one.tile([1, WAY], F32, tag="pinv")
            pinv_b = sb_one.tile([NQ, WAY], F32, tag="pinv_b")
            qminv = sb_one.tile([1, NQ], F32, tag="qminv")
            qminv_col = sb_one.tile([NQ, 1], F32, tag="qminv_col")
            qcol = sb_one.tile([CP, NM], F32, tag="qcol")
            seg_sb = sb_one.tile([CP, NM * NQ], F32, tag="seg_sb")
            dn4r = sb_one.tile([NQ, WAY], F32, tag="dn4r")
            dn4_sb = sb_one.tile([NQ, WAY], F32, tag="dn4_sb")
            cos_sb = sb_one.tile([NQ, WAY], F32, tag="cos_sb")

            # ---- load inputs (already bf16 on host) ----
            for ci in range(NCH):
                nc.sync.dma_start(s_bf[ci][:], xs[ci])
            for ci in range(NCH):
                nc.sync.dma_start(q_bf[ci][:], xq[ci])
            nc.sync.dma_start(seg_sb[:], seg[:, :])

            # ---------------- prep phase (own SBUF/PSUM scope) ----------------
            with ExitStack() as pctx:
                sb_scratch = pctx.enter_context(tc.tile_pool(name="sb_scr", bufs=2))
                sb_prep = pctx.enter_context(tc.tile_pool(name="sb_prep", bufs=1))
                ps_acc = pctx.enter_context(
                    tc.tile_pool(name="ps_acc", bufs=1, space="PSUM")
                )
                ps_bcast = pctx.enter_context(
                    tc.tile_pool(name="ps_bcast", bufs=2, space="PSUM")
                )
                ps_qmat = pctx.enter_context(
                    tc.tile_pool(name="ps_qmat", bufs=1, space="PSUM")
                )

                sinv_row = sb_prep.tile([1, WAY * Y], F32, tag="sinv_row")
                qinv2_row = sb_prep.tile([1, NQ * HWDIM], F32, tag="qinv2_row")
                sinv_b = sb_prep.tile([CP, WAY * Y], BF16, tag="sinv_b")

                # -- support: pooled sums for the cosine path (raw features;
                #    cosine is invariant to the positive mean scaling) --
                for ci in range(NCH):
                    nc.vector.reduce_sum(
                        smean[:, ci * 25 : (ci + 1) * 25],
                        s_bf[ci][:].rearrange("p (n k) -> p n k", k=HWDIM),
                        AX.X,
                    )
                for ci in range(NCH):
                    nc.vector.reduce_sum(
                        proto_t[:, ci * WAY : (ci + 1) * WAY],
                        smean[:, ci * 25 : (ci + 1) * 25].rearrange(
                            "p (n k) -> p n k", k=SHOT
                        ),
                        AX.X,
                    )

                # -- support: per-descriptor L2 norms -> sinv broadcast --
                ssq = ps_acc.tile([1, 5, 512], F32, tag="acc")
                for ci in range(NCH):
                    sqt = sb_scratch.tile([CP, WAY * Y], BF16, tag="sq")
                    nc.scalar.square(sqt[:], s_bf[ci][:])
                    for j in range(5):
                        nc.tensor.matmul(
                            ssq[0:1, j, 0:500],
                            ones_col[:],
                            sqt[:, j * 500 : (j + 1) * 500],
                            start=(ci == 0),
                            stop=(ci == NCH - 1),
                        )
                for j in range(5):
                    nc.vector.reciprocal(
                        sinv_row[:, j * 500 : (j + 1) * 500], ssq[0:1, j, 0:500]
                    )
                    nc.scalar.activation(
                        sinv_row[:, j * 500 : (j + 1) * 500],
                        sinv_row[:, j * 500 : (j + 1) * 500],
                        AF.Sqrt,
                    )
                for j in range(5):
                    bc = ps_bcast.tile([CP, 500], F32, tag="bc")
                    nc.tensor.matmul(
                        bc[:],
                        ones_row[0:1, :],
                        sinv_row[:, j * 500 : (j + 1) * 500],
                        start=True,
                        stop=True,
                    )
                    nc.scalar.copy(sinv_b[:, j * 500 : (j + 1) * 500], bc[:])
                # scale support descriptors in place (all-bf16 DVE fast path)
                for ci in range(NCH):
                    nc.vector.tensor_mul(s_bf[ci][:], s_bf[ci][:], sinv_b[:])

                # -- support: prototype norms (cosine path) --
                sqp = sb_scratch.tile([CP, WAY * Y], BF16, tag="sq")
                nc.scalar.square(sqp[:, 0 : NCH * WAY], proto_t[:])
                psq = ps_acc.tile([1, 5, 512], F32, tag="acc")
                for ci in range(NCH):
                    nc.tensor.matmul(
                        psq[0:1, 0, 0:WAY],
                        ones_col[:],
                        sqp[:, ci * WAY : (ci + 1) * WAY],
                        start=(ci == 0),
                        stop=(ci == NCH - 1),
                    )
                nc.vector.reciprocal(pinv[:], psq[0:1, 0, 0:WAY])
                nc.scalar.activation(pinv[:], pinv[:], AF.Sqrt)
                bcp = ps_bcast.tile([CP, 500], F32, tag="bc")
                nc.tensor.matmul(
                    bcp[0:NQ, 0:WAY],
                    ones_row[0:1, 0:NQ],
                    pinv[:],
                    start=True,
                    stop=True,
                )
                nc.scalar.copy(pinv_b[:], bcp[0:NQ, 0:WAY])

                # -- query: pooled sums + norms (cosine path) --
                for ci in range(NCH):
                    nc.vector.reduce_sum(
                        qm_t[:, ci * NQ : (ci + 1) * NQ],
                        q_bf[ci][:].rearrange("p (n k) -> p n k", k=HWDIM),
                        AX.X,
                    )
                sqm = sb_scratch.tile([CP, WAY * Y], BF16, tag="sq")
                nc.scalar.square(sqm[:, 0 : NCH * NQ], qm_t[:])
                qmsq = ps_acc.tile([1, 5, 512], F32, tag="acc")
                for ci in range(NCH):
                    nc.tensor.matmul(
                        qmsq[0:1, 0, 0:NQ],
                        ones_col[:],
                        sqm[:, ci * NQ : (ci + 1) * NQ],
                        start=(ci == 0),
                        stop=(ci == NCH - 1),
                    )
                nc.vector.reciprocal(qminv[:], qmsq[0:1, 0, 0:NQ])
                nc.scalar.activation(qminv[:], qminv[:], AF.Sqrt)
                qmc = ps_qmat.tile([HWDIM, NQ], F32, tag="qmat")
                nc.tensor.matmul(
                    qmc[0:NQ, 0:1], qminv[:], ones_row[0:1, 0:1], start=True, stop=True
                )
                nc.scalar.copy(qminv_col[:], qmc[0:NQ, 0:1])

                # -- query: per-descriptor norms -> packed 1/||q|| columns --
                qsq = ps_acc.tile([1, 5, 512], F32, tag="acc")
                for ci in range(NCH):
                    sqq = sb_scratch.tile([CP, WAY * Y], BF16, tag="sq")
                    nc.scalar.square(sqq[:, 0 : NQ * HWDIM], q_bf[ci][:])
                    for j in range(5):
                        nc.tensor.matmul(
                            qsq[0:1, j, 0:500],
                            ones_col[:],
                            sqq[:, j * 500 : (j + 1) * 500],
                            start=(ci == 0),
                            stop=(ci == NCH - 1),
                        )
                for j in range(5):
                    nc.vector.reciprocal(
                        qinv2_row[:, j * 500 : (j + 1) * 500], qsq[0:1, j, 0:500]
                    )
                    nc.scalar.activation(
                        qinv2_row[:, j * 500 : (j + 1) * 500],
                        qinv2_row[:, j * 500 : (j + 1) * 500],
                        AF.Sqrt,
                    )
                qmp = ps_qmat.tile([CP, NM], F32, tag="qmat")
                nc.vector.memset(qmp[:], 0.0)
                for m in range(NM):
                    r = _rows(m)
                    nc.tensor.matmul(
                        qmp[0:r, m : m + 1],
                        qinv2_row[:, m * CP : m * CP + r],
                        ones_row[0:1, 0:1],
                        start=True,
                        stop=True,
                    )
                nc.scalar.copy(qcol[:], qmp[:])

            # ---------------- main phase ----------------
            with ExitStack() as mctx:
                ps_sims = mctx.enter_context(
                    tc.tile_pool(name="ps_sims", bufs=5, space="PSUM")
                )
                ps_dn4 = mctx.enter_context(
                    tc.tile_pool(name="ps_dn4", bufs=2, space="PSUM")
                )
                ps_dots = mctx.enter_context(
                    tc.tile_pool(name="ps_dots", bufs=1, space="PSUM")
                )

                # cosine logits
                dots = ps_dots.tile([NQ, WAY], F32, tag="dots")
                for ci in range(NCH):
                    nc.tensor.matmul(
                        dots[:],
                        qm_t[:, ci * NQ : (ci + 1) * NQ],
                        proto_t[:, ci * WAY : (ci + 1) * WAY],
                        start=(ci == 0),
                        stop=(ci == NCH - 1),
                    )
                nc.vector.scalar_tensor_tensor(
                    cos_sb[:],
                    dots[:],
                    qminv_col[:],
                    pinv_b[:],
                    AluOpType.mult,
                    AluOpType.mult,
                )
                nc.sync.dma_start(out_cos[:, :], cos_sb[:])

                # DN4 logits: packed-M sweep over the 2500 query descriptors.
                # dn4ps[q', (w,k)] accumulates seg^T @ (qinv[x] * top8[:, :5])^2
                # across all 20 M-chunks; the per-way k-sum and sqrt run once.
                dn4ps = ps_dn4.tile([NQ, NQ], F32, tag="dn4acc")
                for m in range(NM):
                    r = _rows(m)
                    m8a = sb_loop.tile([CP, WAY * 8], F32, tag="m8a")
                    for w in range(WAY):
                        ps = ps_sims.tile([CP, Y], F32, tag="sims")
                        for ci in range(NCH):
                            nc.tensor.matmul(
                                ps[0:r, :],
                                q_bf[ci][:, m * CP : m * CP + r],
                                s_bf[ci][:, w * Y : (w + 1) * Y],
                                start=(ci == 0),
                                stop=(ci == NCH - 1),
                            )
                        if MAX_FROM_PSUM:
                            nc.vector.max(m8a[0:r, w * 8 : (w + 1) * 8], ps[0:r, :])
                        else:
                            sco = sb_loop.tile([CP, Y], F32, tag="sco")
                            nc.scalar.copy(sco[0:r, :], ps[0:r, :])
                            nc.vector.max(m8a[0:r, w * 8 : (w + 1) * 8], sco[0:r, :])
                    v = sb_loop.tile([CP, WAY * KNN], F32, tag="v")
                    nc.scalar.activation(
                        v[0:r, :],
                        m8a[0:r, :].rearrange("p (w k) -> p w k", k=8)[:, :, 0:KNN],
                        AF.Square,
                        scale=qcol[0:r, m : m + 1],
                    )
                    nc.tensor.matmul(
                        dn4ps[:],
                        seg_sb[0:r, m * NQ : (m + 1) * NQ],
                        v[0:r, :],
                        start=(m == 0),
                        stop=(m == NM - 1),
                        skip_group_check=True,
                    )
                nc.vector.tensor_reduce(
                    dn4r[:],
                    dn4ps[:].rearrange("p (w k) -> p w k", k=KNN),
                    AX.X,
                    AluOpType.add,
                )
                nc.scalar.activation(dn4_sb[:], dn4r[:], AF.Sqrt, scale=DN4_SCALE)
                nc.sync.dma_start(out_dn4[:], dn4_sb[:])

    nc.finalize()
    return nc


_NC_CACHE = None


def _get_program():
    global _NC_CACHE
    if _NC_CACHE is None:
        _NC_CACHE = build_program()
    return _NC_CACHE


def _make_seg():
    seg = np.zeros((CP, NM * NQ), np.float32)
    for m in range(NM):
        for p in range(_rows(m)):
            x = m * CP + p
            seg[p, m * NQ + x // HWDIM] = 1.0
    return seg


def make_in_maps(x_shot, x_query):
    import ml_dtypes

    bf = np.dtype(ml_dtypes.bfloat16)
    x_shot = np.asarray(x_shot, dtype=np.float32).astype(bf)
    x_query = np.asarray(x_query, dtype=np.float32).astype(bf)
    seg = _make_seg()
    in_maps = []
    for core in range(8):
        b, h = divmod(core, 2)
        xs_r = (
            x_shot[b]
            .reshape(WAY * SHOT, NCH, CP, HWDIM)
            .transpose(1, 2, 0, 3)
            .reshape(NCH, CP, WAY * Y)
        )
        xq_r = (
            x_query[b, h * NQ : (h + 1) * NQ]
            .reshape(NQ, NCH, CP, HWDIM)
            .transpose(1, 2, 0, 3)
            .reshape(NCH, CP, NQ * HWDIM)
        )
        in_maps.append(
            {
                "xs": np.ascontiguousarray(xs_r),
                "xq": np.ascontiguousarray(xq_r),
                "seg": seg,
            }
        )
    return in_maps


def assemble(results):
    cos = np.zeros((B, 50, WAY), np.float32)
    dn4 = np.zeros((B, 50, WAY), np.float32)
    for core in range(8):
        b, h = divmod(core, 2)
        cos[b, h * NQ : (h + 1) * NQ] = results[core]["out_cos"]
        dn4[b, h * NQ : (h + 1) * NQ] = results[core]["out_dn4"].reshape(NQ, WAY)
    return cos, dn4


def kernel(x_shot, x_query, r_cos, r_dn4, neighbor_k=5):
    from concourse.bass_utils import run_bass_kernel_spmd

    nc = _get_program()
    in_maps = make_in_maps(x_shot, x_query)
    r = run_bass_kernel_spmd(nc, in_maps, list(range(8)))
    cos, dn4 = assemble(r.results)
    return (
        cos,
        dn4,
        np.asarray(r_cos, np.float32),
        np.asarray(r_dn4, np.float32),
    )


# revision 39
# speedup vs baseline: 1.3380x; 1.1115x over previous
"""DN4/MetaBaseline few-shot head on 8 Trainium2 cores.

Problem shapes (hardcoded): x_shot [4,5,5,640,10,10], x_query [4,50,640,10,10].
Sharding: core i handles episode b = i//2 and query half h = i%2 (25 queries).
Outputs: logits_cos [4,50,5], logits_dn4 [4,50,5], plus r_cos/r_dn4 passthrough.

Per-core pipeline:
  - inputs shipped bf16 (norm/cosine paths) plus a host-packed fp8e4m3 copy of
    the query descriptors in the contiguous [K,2,M]-blocked layout DoubleRow
    LDWEIGHTS requires,
  - support/query descriptor L2 norms via bf16 squares + ones-column matmul
    (partition-dim sum over C) + DVE reciprocal + ACT sqrt,
  - support descriptors scaled by 1/||s|| and rounded to fp8 on gpsimd,
  - sims = q_desc^T s_desc as 300 fp8 DoubleRow matmuls
    [K=2x128, M=128, N=500] into PSUM (way-outer loop so scaling of way w+1
    overlaps matmuls of way w),
  - top-5 per row via the DVE max (top-8) instruction straight from PSUM,
  - per-row 1/||q||^2 folded into the ACT Square as a per-partition scale,
  - ragged per-query reduction via one constant 0/1 segment-matrix matmul
    accumulated across the 20 M-chunks, then k-sum + sqrt,
  - cosine logits from bf16 pooled sums (scale-invariant) + tiny matmuls.
"""

from contextlib import ExitStack

import numpy as np

import concourse.bacc as bacc
import concourse.tile as tile
from concourse import mybir
from concourse.alu_op_type import AluOpType

F32 = mybir.dt.float32
BF16 = mybir.dt.bfloat16
FP8 = mybir.dt.float8e4
AF = mybir.ActivationFunctionType
AX = mybir.AxisListType

B, WAY, SHOT, C, HWDIM = 4, 5, 5, 640, 100
NQ = 25          # queries per core
NCH = 5          # C chunks of 128
CP = 128
Y = SHOT * HWDIM  # 500 support descriptors per way
KNN = 5
DN4_SCALE = 1.0 / float((KNN * 50) ** 2)  # sqrt(x * scale) == sqrt(x) / (k * hw_q)

MAX_FROM_PSUM = True
ABLATE_SIMS = False   # only 1 of 5 ci-chunk matmuls
ABLATE_MAX = False    # skip the DVE max instruction
ABLATE_MAIN = False   # skip the whole DN4 main loop
USE_FP8 = True        # fp8e4m3 DoubleRow sims matmuls


NM = 20  # ceil(NQ*HWDIM / 128) M-chunks of packed query descriptors


def _rows(m):
    return min(CP, NQ * HWDIM - m * CP)


def build_program(loop_n=None, bodies=1):
    nc = bacc.Bacc("TRN2", target_bir_lowering=False, debug=False)
    xs = nc.dram_tensor("xs", [NCH, CP, WAY * Y], BF16, kind="ExternalInput")
    xq = nc.dram_tensor("xq", [NCH, CP, NQ * HWDIM], BF16, kind="ExternalInput")
    seg = nc.dram_tensor("seg", [CP, NM * NQ], F32, kind="ExternalInput")
    xqf8 = nc.dram_tensor("xqf8", [3, CP, NM * 2 * CP], FP8, kind="ExternalInput")
    out_cos = nc.dram_tensor("out_cos", [NQ, WAY], F32, kind="ExternalOutput")
    out_dn4 = nc.dram_tensor("out_dn4", [NQ * WAY], F32, kind="ExternalOutput")

    with tile.TileContext(nc) as tc:
        with ExitStack() as octx:
            if loop_n is not None:
                octx.enter_context(tc.For_i(0, loop_n, 1))
            for _ in range(bodies):
                _build_body(nc, tc, xs, xq, seg, xqf8, out_cos, out_dn4)

    nc.finalize()
    return nc


def _build_body(nc, tc, xs, xq, seg, xqf8, out_cos, out_dn4):
    if True:
        with ExitStack() as ctx:
            sb_const = ctx.enter_context(tc.tile_pool(name="sb_const", bufs=1))
            sb_bf = ctx.enter_context(tc.tile_pool(name="sb_bf", bufs=NCH))
            sb_one = ctx.enter_context(tc.tile_pool(name="sb_one", bufs=1))
            sb_loop = ctx.enter_context(tc.tile_pool(name="sb_loop", bufs=4))

            ones_row = sb_const.tile([1, CP], F32)
            nc.vector.memset(ones_row[:], 1.0)
            ones_col = sb_const.tile([CP, 1], BF16)
            nc.vector.memset(ones_col[:], 1.0)

            # persistent tiles
            s_bf = [
                sb_bf.tile([CP, WAY * Y], BF16, tag="s_bf", name=f"s_bf{i}")
                for i in range(NCH)
            ]
            q_bf = [
                sb_bf.tile([CP, NQ * HWDIM], BF16, tag="q_bf", name=f"q_bf{i}")
                for i in range(NCH)
            ]
            if USE_FP8:
                s_f8 = [
                    sb_bf.tile([CP, 2, WAY * Y], FP8, tag="s_f8", name=f"s_f8_{i}")
                    for i in range(3)
                ]
                q_f8 = [
                    sb_bf.tile([CP, NM, 2, CP], FP8, tag="q_f8", name=f"q_f8_{i}")
                    for i in range(3)
                ]
            else:
                s_sc = [
                    sb_bf.tile([CP, WAY * Y], BF16, tag="s_sc", name=f"s_sc{i}")
                    for i in range(NCH)
                ]
            smean = sb_one.tile([CP, NCH * WAY * SHOT], BF16, tag="smean")
            proto_t = sb_one.tile([CP, NCH * WAY], BF16, tag="proto")
            qm_t = sb_one.tile([CP, NCH * NQ], BF16, tag="qm")
            pinv = sb_one.tile([1, WAY], F32, tag="pinv")
            pinv_b = sb_one.tile([NQ, WAY], F32, tag="pinv_b")
            qminv = sb_one.tile([1, NQ], F32, tag="qminv")
            qminv_col = sb_one.tile([NQ, 1], F32, tag="qminv_col")
            qcol = sb_one.tile([CP, NM], F32, tag="qcol")
            seg_sb = sb_one.tile([CP, NM * NQ], F32, tag="seg_sb")
            dn4r = sb_one.tile([NQ, WAY], F32, tag="dn4r")
            dn4_sb = sb_one.tile([NQ, WAY], F32, tag="dn4_sb")
            cos_sb = sb_one.tile([NQ, WAY], F32, tag="cos_sb")

            # ---- load inputs (already bf16 on host) ----
            for ci in range(NCH):
                nc.sync.dma_start(s_bf[ci][:], xs[ci])
            for ci in range(NCH):
                nc.sync.dma_start(q_bf[ci][:], xq[ci])
            nc.sync.dma_start(seg_sb[:], seg[:, :])
            if USE_FP8:
                for pi in range(3):
                    nc.sync.dma_start(q_f8[pi][:], xqf8[pi])

            # ---------------- prep phase (own SBUF/PSUM scope) ----------------
            with ExitStack() as pctx:
                sb_scratch = pctx.enter_context(tc.tile_pool(name="sb_scr", bufs=2))
                sb_prep = pctx.enter_context(tc.tile_pool(name="sb_prep", bufs=1))
                ps_acc = pctx.enter_context(
                    tc.tile_pool(name="ps_acc", bufs=1, space="PSUM")
                )
                ps_bcast = pctx.enter_context(
                    tc.tile_pool(name="ps_bcast", bufs=2, space="PSUM")
                )
                ps_qmat = pctx.enter_context(
                    tc.tile_pool(name="ps_qmat", bufs=1, space="PSUM")
                )

                sinv_row = sb_prep.tile([1, WAY * Y], F32, tag="sinv_row")
                qinv2_row = sb_prep.tile([1, NQ * HWDIM], F32, tag="qinv2_row")
                sinv_b = sb_prep.tile([CP, WAY * Y], BF16, tag="sinv_b")

                # -- support: per-descriptor L2 norms -> sinv broadcast --
                ssq = ps_acc.tile([1, 5, 512], F32, tag="acc")
                for ci in range(NCH):
                    sqt = sb_scratch.tile([CP, WAY * Y], BF16, tag="sq")
                    nc.scalar.square(sqt[:], s_bf[ci][:])
                    for j in range(5):
                        nc.tensor.matmul(
                            ssq[0:1, j, 0:500],
                            ones_col[:],
                            sqt[:, j * 500 : (j + 1) * 500],
                            start=(ci == 0),
                            stop=(ci == NCH - 1),
                        )
                for j in range(5):
                    nc.vector.reciprocal(
                        sinv_row[:, j * 500 : (j + 1) * 500], ssq[0:1, j, 0:500]
                    )
                    nc.scalar.activation(
                        sinv_row[:, j * 500 : (j + 1) * 500],
                        sinv_row[:, j * 500 : (j + 1) * 500],
                        AF.Sqrt,
                    )
                for j in range(5):
                    bc = ps_bcast.tile([CP, 500], F32, tag="bc")
                    nc.tensor.matmul(
                        bc[:],
                        ones_row[0:1, :],
                        sinv_row[:, j * 500 : (j + 1) * 500],
                        start=True,
                        stop=True,
                    )
                    nc.scalar.copy(sinv_b[:, j * 500 : (j + 1) * 500], bc[:])
                # scale support descriptors, j-sliced so the first way's
                # columns unblock early
                for j in range(5):
                    for ci in range(NCH):
                        sl = slice(j * 500, (j + 1) * 500)
                        if USE_FP8:
                            nc.gpsimd.tensor_mul(
                                s_f8[ci // 2][:, ci % 2, sl],
                                s_bf[ci][:, sl],
                                sinv_b[:, sl],
                            )
                        else:
                            nc.vector.tensor_mul(
                                s_sc[ci][:, sl], s_bf[ci][:, sl], sinv_b[:, sl]
                            )
                if USE_FP8:
                    # zero the padded 6th K-chunk of the support pairs
                    nc.vector.memset(s_f8[2][:, 1, :], 0.0)

                # -- support/query pooled sums for the cosine path (raw
                #    features; cosine is invariant to the mean scaling) --
                with nc.allow_low_precision("cosine pooled sums; rounded once"):
                    for ci in range(NCH):
                        nc.vector.reduce_sum(
                            smean[:, ci * 25 : (ci + 1) * 25],
                            s_bf[ci][:].rearrange("p (n k) -> p n k", k=HWDIM),
                            AX.X,
                        )
                    for ci in range(NCH):
                        nc.vector.reduce_sum(
                            proto_t[:, ci * WAY : (ci + 1) * WAY],
                            smean[:, ci * 25 : (ci + 1) * 25].rearrange(
                                "p (n k) -> p n k", k=SHOT
                            ),
                            AX.X,
                        )

                # -- support: prototype norms (cosine path) --
                sqp = sb_scratch.tile([CP, WAY * Y], BF16, tag="sq")
                nc.scalar.square(sqp[:, 0 : NCH * WAY], proto_t[:])
                psq = ps_acc.tile([1, 5, 512], F32, tag="acc")
                for ci in range(NCH):
                    nc.tensor.matmul(
                        psq[0:1, 0, 0:WAY],
                        ones_col[:],
                        sqp[:, ci * WAY : (ci + 1) * WAY],
                        start=(ci == 0),
                        stop=(ci == NCH - 1),
                    )
                nc.vector.reciprocal(pinv[:], psq[0:1, 0, 0:WAY])
                nc.scalar.activation(pinv[:], pinv[:], AF.Sqrt)
                bcp = ps_bcast.tile([CP, 500], F32, tag="bc")
                nc.tensor.matmul(
                    bcp[0:NQ, 0:WAY],
                    ones_row[0:1, 0:NQ],
                    pinv[:],
                    start=True,
                    stop=True,
                )
                nc.scalar.copy(pinv_b[:], bcp[0:NQ, 0:WAY])

                # -- query: pooled sums + norms (cosine path) --
                with nc.allow_low_precision("cosine pooled sums; rounded once"):
                    for ci in range(NCH):
                        nc.vector.reduce_sum(
                            qm_t[:, ci * NQ : (ci + 1) * NQ],
                            q_bf[ci][:].rearrange("p (n k) -> p n k", k=HWDIM),
                            AX.X,
                        )
                sqm = sb_scratch.tile([CP, WAY * Y], BF16, tag="sq")
                nc.scalar.square(sqm[:, 0 : NCH * NQ], qm_t[:])
                qmsq = ps_acc.tile([1, 5, 512], F32, tag="acc")
                for ci in range(NCH):
                    nc.tensor.matmul(
                        qmsq[0:1, 0, 0:NQ],
                        ones_col[:],
                        sqm[:, ci * NQ : (ci + 1) * NQ],
                        start=(ci == 0),
                        stop=(ci == NCH - 1),
                    )
                nc.vector.reciprocal(qminv[:], qmsq[0:1, 0, 0:NQ])
                nc.scalar.activation(qminv[:], qminv[:], AF.Sqrt)
                qmc = ps_qmat.tile([HWDIM, NQ], F32, tag="qmat")
                nc.tensor.matmul(
                    qmc[0:NQ, 0:1], qminv[:], ones_row[0:1, 0:1], start=True, stop=True
                )
                nc.scalar.copy(qminv_col[:], qmc[0:NQ, 0:1])

                # -- query: per-descriptor norms -> packed 1/||q|| columns --
                qsq = ps_acc.tile([1, 5, 512], F32, tag="acc")
                for ci in range(NCH):
                    sqq = sb_scratch.tile([CP, WAY * Y], BF16, tag="sq")
                    nc.scalar.square(sqq[:, 0 : NQ * HWDIM], q_bf[ci][:])
                    for j in range(5):
                        nc.tensor.matmul(
                            qsq[0:1, j, 0:500],
                            ones_col[:],
                            sqq[:, j * 500 : (j + 1) * 500],
                            start=(ci == 0),
                            stop=(ci == NCH - 1),
                        )
                for j in range(5):
                    nc.vector.reciprocal(
                        qinv2_row[:, j * 500 : (j + 1) * 500], qsq[0:1, j, 0:500]
                    )
                    nc.scalar.activation(
                        qinv2_row[:, j * 500 : (j + 1) * 500],
                        qinv2_row[:, j * 500 : (j + 1) * 500],
                        AF.Sqrt,
                    )
                qmp = ps_qmat.tile([CP, NM], F32, tag="qmat")
                nc.vector.memset(qmp[:], 0.0)
                for m in range(NM):
                    r = _rows(m)
                    nc.tensor.matmul(
                        qmp[0:r, m : m + 1],
                        qinv2_row[:, m * CP : m * CP + r],
                        ones_row[0:1, 0:1],
                        start=True,
                        stop=True,
                    )
                nc.scalar.copy(qcol[:], qmp[:])

            # ---------------- main phase ----------------
            with ExitStack() as mctx:
                ps_sims = mctx.enter_context(
                    tc.tile_pool(name="ps_sims", bufs=5, space="PSUM")
                )
                ps_dn4 = mctx.enter_context(
                    tc.tile_pool(name="ps_dn4", bufs=2, space="PSUM")
                )
                ps_dots = mctx.enter_context(
                    tc.tile_pool(name="ps_dots", bufs=1, space="PSUM")
                )

                # cosine logits
                dots = ps_dots.tile([NQ, WAY], F32, tag="dots")
                for ci in range(NCH):
                    nc.tensor.matmul(
                        dots[:],
                        qm_t[:, ci * NQ : (ci + 1) * NQ],
                        proto_t[:, ci * WAY : (ci + 1) * WAY],
                        start=(ci == 0),
                        stop=(ci == NCH - 1),
                    )
                nc.vector.scalar_tensor_tensor(
                    cos_sb[:],
                    dots[:],
                    qminv_col[:],
                    pinv_b[:],
                    AluOpType.mult,
                    AluOpType.mult,
                )
                nc.sync.dma_start(out_cos[:, :], cos_sb[:])

                # DN4 logits: packed-M sweep over the 2500 query descriptors.
                # dn4ps[q', (w,k)] accumulates seg^T @ (qinv[x] * top8[:, :5])^2
                # across all 20 M-chunks; the per-way k-sum and sqrt run once.
                if ABLATE_MAIN:
                    return
                dn4ps = ps_dn4.tile([NQ, NQ], F32, tag="dn4acc")
                m8a_tiles = [
                    sb_loop.tile(
                        [CP, WAY * 8], F32, tag="m8a", name=f"m8a_{m}", bufs=NM
                    )
                    for m in range(NM)
                ]
                for w in range(WAY):
                    last_way = w == WAY - 1
                    for m in range(NM):
                        r = _rows(m)
                        ps = ps_sims.tile([CP, Y], F32, tag="sims")
                        if USE_FP8:
                            npair = 1 if ABLATE_SIMS else 3
                            for pi in range(npair):
                                nc.tensor.matmul(
                                    ps[:, :],
                                    q_f8[pi][:, m, :, :],
                                    s_f8[pi][:, :, w * Y : (w + 1) * Y],
                                    start=(pi == 0),
                                    stop=(pi == npair - 1),
                                    perf_mode=mybir.MatmulPerfMode.DoubleRow,
                                )
                        else:
                            nch_eff = 1 if ABLATE_SIMS else NCH
                            for ci in range(nch_eff):
                                nc.tensor.matmul(
                                    ps[0:r, :],
                                    q_bf[ci][:, m * CP : m * CP + r],
                                    s_sc[ci][:, w * Y : (w + 1) * Y],
                                    start=(ci == 0),
                                    stop=(ci == nch_eff - 1),
                                )
                        m8a = m8a_tiles[m]
                        if ABLATE_MAX:
                            nc.vector.memset(m8a[0:r, w * 8 : (w + 1) * 8], 0.5)
                        elif MAX_FROM_PSUM:
                            nc.vector.max(m8a[0:r, w * 8 : (w + 1) * 8], ps[0:r, :])
                        else:
                            sco = sb_loop.tile([CP, Y], F32, tag="sco")
                            nc.scalar.copy(sco[0:r, :], ps[0:r, :])
                            nc.vector.max(m8a[0:r, w * 8 : (w + 1) * 8], sco[0:r, :])
                        if last_way:
                            r2 = _rows(m)
                            v = sb_loop.tile([CP, WAY * KNN], F32, tag="v")
                            nc.scalar.activation(
                                v[0:r2, :],
                                m8a[0:r2, :].rearrange("p (w k) -> p w k", k=8)[
                                    :, :, 0:KNN
                                ],
                                AF.Square,
                                scale=qcol[0:r2, m : m + 1],
                            )
                            nc.tensor.matmul(
                                dn4ps[:],
                                seg_sb[0:r2, m * NQ : (m + 1) * NQ],
                                v[0:r2, :],
                                start=(m == 0),
                                stop=(m == NM - 1),
                                skip_group_check=True,
                            )
                nc.vector.tensor_reduce(
                    dn4r[:],
                    dn4ps[:].rearrange("p (w k) -> p w k", k=KNN),
                    AX.X,
                    AluOpType.add,
                )
                nc.scalar.activation(dn4_sb[:], dn4r[:], AF.Sqrt, scale=DN4_SCALE)
                nc.sync.dma_start(out_dn4[:], dn4_sb[:])


_NC_CACHE = None


def _get_program():
    global _NC_CACHE
    if _NC_CACHE is None:
        _NC_CACHE = build_program()
    return _NC_CACHE


def _make_seg():
    seg = np.zeros((CP, NM * NQ), np.float32)
    for m in range(NM):
        for p in range(_rows(m)):
            x = m * CP + p
            seg[p, m * NQ + x // HWDIM] = 1.0
    return seg


def _pack_q_fp8(xq_r):
    # xq_r: [NCH, CP, NQ*HWDIM] float32 -> [3, CP, NM*2*CP] fp8e4m3 with the
    # descriptor dim padded 2500->2560 and a zero 6th K-chunk.
    import ml_dtypes

    f8 = np.dtype(ml_dtypes.float8_e4m3)
    out = np.zeros((3, CP, NM, 2, CP), np.float32)
    padded = np.zeros((NCH, CP, NM * CP), np.float32)
    padded[:, :, : NQ * HWDIM] = xq_r
    for ci in range(NCH):
        out[ci // 2, :, :, ci % 2, :] = padded[ci].reshape(CP, NM, CP)
    return np.ascontiguousarray(out.astype(f8).reshape(3, CP, NM * 2 * CP))


def make_in_maps(x_shot, x_query):
    import ml_dtypes

    bf = np.dtype(ml_dtypes.bfloat16)
    x_shot = np.asarray(x_shot, dtype=np.float32).astype(bf)
    x_query = np.asarray(x_query, dtype=np.float32).astype(bf)
    seg = _make_seg()
    in_maps = []
    for core in range(8):
        b, h = divmod(core, 2)
        xs_r = (
            x_shot[b]
            .reshape(WAY * SHOT, NCH, CP, HWDIM)
            .transpose(1, 2, 0, 3)
            .reshape(NCH, CP, WAY * Y)
        )
        xq_r = (
            x_query[b, h * NQ : (h + 1) * NQ]
            .reshape(NQ, NCH, CP, HWDIM)
            .transpose(1, 2, 0, 3)
            .reshape(NCH, CP, NQ * HWDIM)
        )
        m = {
            "xs": np.ascontiguousarray(xs_r),
            "xq": np.ascontiguousarray(xq_r),
            "seg": seg,
        }
        if USE_FP8:
            m["xqf8"] = _pack_q_fp8(xq_r.astype(np.float32))
        in_maps.append(m)
    return in_maps


def assemble(results):
    cos = np.zeros((B, 50, WAY), np.float32)
    dn4 = np.zeros((B, 50, WAY), np.float32)
    for core in range(8):
        b, h = divmod(core, 2)
        cos[b, h * NQ : (h + 1) * NQ] = results[core]["out_cos"]
        dn4[b, h * NQ : (h + 1) * NQ] = results[core]["out_dn4"].reshape(NQ, WAY)
    return cos, dn4


def kernel(x_shot, x_query, r_cos, r_dn4, neighbor_k=5):
    from concourse.bass_utils import run_bass_kernel_spmd

    nc = _get_program()
    in_maps = make_in_maps(x_shot, x_query)
    r = run_bass_kernel_spmd(nc, in_maps, list(range(8)))
    cos, dn4 = assemble(r.results)
    return (
        cos,
        dn4,
        np.asarray(r_cos, np.float32),
        np.asarray(r_dn4, np.float32),
    )


# revision 42
# speedup vs baseline: 1.6595x; 1.2403x over previous
"""DN4/MetaBaseline few-shot head on 8 Trainium2 cores.

Problem shapes (hardcoded): x_shot [4,5,5,640,10,10], x_query [4,50,640,10,10].
Sharding: core i handles episode b = i//2 and query half h = i%2 (25 queries).
Outputs: logits_cos [4,50,5], logits_dn4 [4,50,5], plus r_cos/r_dn4 passthrough.

Per-core pipeline:
  - inputs shipped bf16 (norm/cosine paths) plus a host-packed fp8e4m3 copy of
    the query descriptors in the contiguous [K,2,M]-blocked layout DoubleRow
    LDWEIGHTS requires,
  - support/query descriptor L2 norms via bf16 squares + ones-column matmul
    (partition-dim sum over C) + DVE reciprocal + ACT sqrt,
  - support descriptors scaled by 1/||s|| and rounded to fp8 on gpsimd,
  - sims = q_desc^T s_desc as 300 fp8 DoubleRow matmuls
    [K=2x128, M=128, N=500] into PSUM (way-outer loop so scaling of way w+1
    overlaps matmuls of way w),
  - top-5 per row via the DVE max (top-8) instruction straight from PSUM,
  - per-row 1/||q||^2 folded into the ACT Square as a per-partition scale,
  - ragged per-query reduction via one constant 0/1 segment-matrix matmul
    accumulated across the 20 M-chunks, then k-sum + sqrt,
  - cosine logits from bf16 pooled sums (scale-invariant) + tiny matmuls.
"""

from contextlib import ExitStack

import numpy as np

import concourse.bacc as bacc
import concourse.tile as tile
from concourse import mybir
from concourse.alu_op_type import AluOpType

F32 = mybir.dt.float32
BF16 = mybir.dt.bfloat16
FP8 = mybir.dt.float8e4
AF = mybir.ActivationFunctionType
AX = mybir.AxisListType

B, WAY, SHOT, C, HWDIM = 4, 5, 5, 640, 100
NQ = 25          # queries per core
NCH = 5          # C chunks of 128
CP = 128
Y = SHOT * HWDIM  # 500 support descriptors per way
KNN = 5
DN4_SCALE = 1.0 / float((KNN * 50) ** 2)  # sqrt(x * scale) == sqrt(x) / (k * hw_q)

MAX_FROM_PSUM = True
ABLATE_SIMS = False   # only 1 of 5 ci-chunk matmuls
ABLATE_MAX = False    # skip the DVE max instruction
ABLATE_MAIN = False   # skip the whole DN4 main loop
USE_FP8 = True        # fp8e4m3 DoubleRow sims matmuls


NM = 20  # ceil(NQ*HWDIM / 128) M-chunks of packed query descriptors


def _rows(m):
    return min(CP, NQ * HWDIM - m * CP)


def build_program(loop_n=None, bodies=1):
    nc = bacc.Bacc("TRN2", target_bir_lowering=False, debug=False)
    xs = nc.dram_tensor("xs", [NCH, CP, WAY * Y], BF16, kind="ExternalInput")
    xq = nc.dram_tensor("xq", [NCH, CP, NQ * HWDIM], BF16, kind="ExternalInput")
    seg = nc.dram_tensor("seg", [CP, NM * NQ], F32, kind="ExternalInput")
    xqf8 = nc.dram_tensor("xqf8", [3, CP, NM * 2 * CP], FP8, kind="ExternalInput")
    out_cos = nc.dram_tensor("out_cos", [NQ, WAY], F32, kind="ExternalOutput")
    out_dn4 = nc.dram_tensor("out_dn4", [NQ * WAY], F32, kind="ExternalOutput")

    with tile.TileContext(nc) as tc:
        with ExitStack() as octx:
            if loop_n is not None:
                octx.enter_context(tc.For_i(0, loop_n, 1))
            for _ in range(bodies):
                _build_body(nc, tc, xs, xq, seg, xqf8, out_cos, out_dn4)

    nc.finalize()
    return nc


def _build_body(nc, tc, xs, xq, seg, xqf8, out_cos, out_dn4):
    if True:
        with ExitStack() as ctx:
            sb_const = ctx.enter_context(tc.tile_pool(name="sb_const", bufs=1))
            sb_bf = ctx.enter_context(tc.tile_pool(name="sb_bf", bufs=NCH))
            sb_one = ctx.enter_context(tc.tile_pool(name="sb_one", bufs=1))
            sb_loop = ctx.enter_context(tc.tile_pool(name="sb_loop", bufs=4))

            ones_row = sb_const.tile([1, CP], F32)
            nc.vector.memset(ones_row[:], 1.0)
            ones_col = sb_const.tile([CP, 1], BF16)
            nc.vector.memset(ones_col[:], 1.0)

            # persistent tiles
            s_bf = [
                sb_bf.tile([CP, WAY * Y], BF16, tag="s_bf", name=f"s_bf{i}")
                for i in range(NCH)
            ]
            q_bf = [
                sb_bf.tile([CP, NQ * HWDIM], BF16, tag="q_bf", name=f"q_bf{i}")
                for i in range(NCH)
            ]
            if USE_FP8:
                s_f8 = [
                    sb_bf.tile([CP, 2, WAY * Y], FP8, tag="s_f8", name=f"s_f8_{i}")
                    for i in range(3)
                ]
                q_f8 = [
                    sb_bf.tile([CP, NM, 2, CP], FP8, tag="q_f8", name=f"q_f8_{i}")
                    for i in range(3)
                ]
            else:
                s_sc = [
                    sb_bf.tile([CP, WAY * Y], BF16, tag="s_sc", name=f"s_sc{i}")
                    for i in range(NCH)
                ]
            smean = sb_one.tile([CP, NCH * WAY * SHOT], BF16, tag="smean")
            proto_t = sb_one.tile([CP, NCH * WAY], BF16, tag="proto")
            qm_t = sb_one.tile([CP, NCH * NQ], BF16, tag="qm")
            pinv = sb_one.tile([1, WAY], F32, tag="pinv")
            pinv_b = sb_one.tile([NQ, WAY], F32, tag="pinv_b")
            qminv = sb_one.tile([1, NQ], F32, tag="qminv")
            qminv_col = sb_one.tile([NQ, 1], F32, tag="qminv_col")
            qcol = sb_one.tile([CP, NM], F32, tag="qcol")
            seg_sb = sb_one.tile([CP, NM * NQ], F32, tag="seg_sb")
            dn4r = sb_one.tile([NQ, WAY], F32, tag="dn4r")
            dn4_sb = sb_one.tile([NQ, WAY], F32, tag="dn4_sb")
            cos_sb = sb_one.tile([NQ, WAY], F32, tag="cos_sb")

            # ---- load inputs (already bf16 on host) ----
            for ci in range(NCH):
                nc.sync.dma_start(s_bf[ci][:], xs[ci])
            for ci in range(NCH):
                nc.sync.dma_start(q_bf[ci][:], xq[ci])
            nc.sync.dma_start(seg_sb[:], seg[:, :])
            if USE_FP8:
                for pi in range(3):
                    nc.sync.dma_start(q_f8[pi][:], xqf8[pi])

            # ---------------- prep phase (own SBUF/PSUM scope) ----------------
            with ExitStack() as pctx:
                sb_scratch = pctx.enter_context(tc.tile_pool(name="sb_scr", bufs=2))
                sb_prep = pctx.enter_context(tc.tile_pool(name="sb_prep", bufs=1))
                ps_acc = pctx.enter_context(
                    tc.tile_pool(name="ps_acc", bufs=1, space="PSUM")
                )
                ps_bcast = pctx.enter_context(
                    tc.tile_pool(name="ps_bcast", bufs=2, space="PSUM")
                )
                ps_qmat = pctx.enter_context(
                    tc.tile_pool(name="ps_qmat", bufs=1, space="PSUM")
                )

                sinv_row = sb_prep.tile([1, WAY * Y], F32, tag="sinv_row")
                qinv2_row = sb_prep.tile([1, NQ * HWDIM], F32, tag="qinv2_row")
                sinv_b = sb_prep.tile([CP, WAY * Y], BF16, tag="sinv_b")

                # -- support: per-descriptor L2 norms -> sinv broadcast --
                ssq = ps_acc.tile([1, 5, 512], F32, tag="acc")
                for ci in range(NCH):
                    sqt = sb_scratch.tile([CP, WAY * Y], BF16, tag="sq")
                    nc.scalar.square(sqt[:], s_bf[ci][:])
                    for j in range(5):
                        nc.tensor.matmul(
                            ssq[0:1, j, 0:500],
                            ones_col[:],
                            sqt[:, j * 500 : (j + 1) * 500],
                            start=(ci == 0),
                            stop=(ci == NCH - 1),
                        )
                for j in range(5):
                    nc.vector.reciprocal(
                        sinv_row[:, j * 500 : (j + 1) * 500], ssq[0:1, j, 0:500]
                    )
                    nc.scalar.activation(
                        sinv_row[:, j * 500 : (j + 1) * 500],
                        sinv_row[:, j * 500 : (j + 1) * 500],
                        AF.Sqrt,
                    )
                for j in range(5):
                    bc = ps_bcast.tile([CP, 500], F32, tag="bc")
                    nc.tensor.matmul(
                        bc[:],
                        ones_row[0:1, :],
                        sinv_row[:, j * 500 : (j + 1) * 500],
                        start=True,
                        stop=True,
                    )
                    nc.scalar.copy(sinv_b[:, j * 500 : (j + 1) * 500], bc[:])
                # scale support descriptors, j-sliced so the first way's
                # columns unblock early
                for j in range(5):
                    for ci in range(NCH):
                        sl = slice(j * 500, (j + 1) * 500)
                        if USE_FP8:
                            nc.gpsimd.tensor_mul(
                                s_f8[ci // 2][:, ci % 2, sl],
                                s_bf[ci][:, sl],
                                sinv_b[:, sl],
                            )
                        else:
                            nc.vector.tensor_mul(
                                s_sc[ci][:, sl], s_bf[ci][:, sl], sinv_b[:, sl]
                            )
                if USE_FP8:
                    # zero the padded 6th K-chunk of the support pairs
                    nc.vector.memset(s_f8[2][:, 1, :], 0.0)

                # -- support/query pooled sums for the cosine path (raw
                #    features; cosine is invariant to the mean scaling) --
                with nc.allow_low_precision("cosine pooled sums; rounded once"):
                    for ci in range(NCH):
                        nc.vector.reduce_sum(
                            smean[:, ci * 25 : (ci + 1) * 25],
                            s_bf[ci][:].rearrange("p (n k) -> p n k", k=HWDIM),
                            AX.X,
                        )
                    for ci in range(NCH):
                        nc.vector.reduce_sum(
                            proto_t[:, ci * WAY : (ci + 1) * WAY],
                            smean[:, ci * 25 : (ci + 1) * 25].rearrange(
                                "p (n k) -> p n k", k=SHOT
                            ),
                            AX.X,
                        )

                # -- support: prototype norms (cosine path) --
                sqp = sb_scratch.tile([CP, WAY * Y], BF16, tag="sq")
                nc.scalar.square(sqp[:, 0 : NCH * WAY], proto_t[:])
                psq = ps_acc.tile([1, 5, 512], F32, tag="acc")
                for ci in range(NCH):
                    nc.tensor.matmul(
                        psq[0:1, 0, 0:WAY],
                        ones_col[:],
                        sqp[:, ci * WAY : (ci + 1) * WAY],
                        start=(ci == 0),
                        stop=(ci == NCH - 1),
                    )
                nc.vector.reciprocal(pinv[:], psq[0:1, 0, 0:WAY])
                nc.scalar.activation(pinv[:], pinv[:], AF.Sqrt)
                bcp = ps_bcast.tile([CP, 500], F32, tag="bc")
                nc.tensor.matmul(
                    bcp[0:NQ, 0:WAY],
                    ones_row[0:1, 0:NQ],
                    pinv[:],
                    start=True,
                    stop=True,
                )
                nc.scalar.copy(pinv_b[:], bcp[0:NQ, 0:WAY])

                # -- query: pooled sums + norms (cosine path) --
                with nc.allow_low_precision("cosine pooled sums; rounded once"):
                    for ci in range(NCH):
                        nc.vector.reduce_sum(
                            qm_t[:, ci * NQ : (ci + 1) * NQ],
                            q_bf[ci][:].rearrange("p (n k) -> p n k", k=HWDIM),
                            AX.X,
                        )
                sqm = sb_scratch.tile([CP, WAY * Y], BF16, tag="sq")
                nc.scalar.square(sqm[:, 0 : NCH * NQ], qm_t[:])
                qmsq = ps_acc.tile([1, 5, 512], F32, tag="acc")
                for ci in range(NCH):
                    nc.tensor.matmul(
                        qmsq[0:1, 0, 0:NQ],
                        ones_col[:],
                        sqm[:, ci * NQ : (ci + 1) * NQ],
                        start=(ci == 0),
                        stop=(ci == NCH - 1),
                    )
                nc.vector.reciprocal(qminv[:], qmsq[0:1, 0, 0:NQ])
                nc.scalar.activation(qminv[:], qminv[:], AF.Sqrt)
                qmc = ps_qmat.tile([HWDIM, NQ], F32, tag="qmat")
                nc.tensor.matmul(
                    qmc[0:NQ, 0:1], qminv[:], ones_row[0:1, 0:1], start=True, stop=True
                )
                nc.scalar.copy(qminv_col[:], qmc[0:NQ, 0:1])

                # -- query: per-descriptor norms -> packed 1/||q|| columns --
                qsq = ps_acc.tile([1, 5, 512], F32, tag="acc")
                for ci in range(NCH):
                    sqq = sb_scratch.tile([CP, WAY * Y], BF16, tag="sq")
                    nc.scalar.square(sqq[:, 0 : NQ * HWDIM], q_bf[ci][:])
                    for j in range(5):
                        nc.tensor.matmul(
                            qsq[0:1, j, 0:500],
                            ones_col[:],
                            sqq[:, j * 500 : (j + 1) * 500],
                            start=(ci == 0),
                            stop=(ci == NCH - 1),
                        )
                for j in range(5):
                    nc.vector.reciprocal(
                        qinv2_row[:, j * 500 : (j + 1) * 500], qsq[0:1, j, 0:500]
                    )
                    nc.scalar.activation(
                        qinv2_row[:, j * 500 : (j + 1) * 500],
                        qinv2_row[:, j * 500 : (j + 1) * 500],
                        AF.Sqrt,
                    )
                qmp = ps_qmat.tile([CP, NM], F32, tag="qmat")
                nc.vector.memset(qmp[:], 0.0)
                for m in range(NM):
                    r = _rows(m)
                    nc.tensor.matmul(
                        qmp[0:r, m : m + 1],
                        qinv2_row[:, m * CP : m * CP + r],
                        ones_row[0:1, 0:1],
                        start=True,
                        stop=True,
                    )
                nc.scalar.copy(qcol[:], qmp[:])

            # ---------------- main phase ----------------
            with ExitStack() as mctx:
                ps_sims = mctx.enter_context(
                    tc.tile_pool(name="ps_sims", bufs=5, space="PSUM")
                )
                ps_dn4 = mctx.enter_context(
                    tc.tile_pool(name="ps_dn4", bufs=2, space="PSUM")
                )
                ps_dots = mctx.enter_context(
                    tc.tile_pool(name="ps_dots", bufs=1, space="PSUM")
                )

                # cosine logits
                dots = ps_dots.tile([NQ, WAY], F32, tag="dots")
                for ci in range(NCH):
                    nc.tensor.matmul(
                        dots[:],
                        qm_t[:, ci * NQ : (ci + 1) * NQ],
                        proto_t[:, ci * WAY : (ci + 1) * WAY],
                        start=(ci == 0),
                        stop=(ci == NCH - 1),
                    )
                nc.vector.scalar_tensor_tensor(
                    cos_sb[:],
                    dots[:],
                    qminv_col[:],
                    pinv_b[:],
                    AluOpType.mult,
                    AluOpType.mult,
                )
                nc.sync.dma_start(out_cos[:, :], cos_sb[:])

                # DN4 logits: packed-M sweep over the 2500 query descriptors.
                # dn4ps[q', (w,k)] accumulates seg^T @ (qinv[x] * top8[:, :5])^2
                # across all 20 M-chunks; the per-way k-sum and sqrt run once.
                if ABLATE_MAIN:
                    return
                dn4ps = ps_dn4.tile([NQ, NQ], F32, tag="dn4acc")
                m8a_tiles = [
                    sb_loop.tile(
                        [CP, WAY * 8], F32, tag="m8a", name=f"m8a_{m}", bufs=NM
                    )
                    for m in range(NM)
                ]
                for w in range(WAY):
                    last_way = w == WAY - 1
                    for m in range(NM):
                        r = _rows(m)
                        ps = ps_sims.tile([CP, Y], F32, tag="sims")
                        if USE_FP8:
                            npair = 1 if ABLATE_SIMS else 3
                            for pi in range(npair):
                                nc.tensor.matmul(
                                    ps[:, :],
                                    q_f8[pi][:, m, :, :],
                                    s_f8[pi][:, :, w * Y : (w + 1) * Y],
                                    start=(pi == 0),
                                    stop=(pi == npair - 1),
                                    perf_mode=mybir.MatmulPerfMode.DoubleRow,
                                )
                        else:
                            nch_eff = 1 if ABLATE_SIMS else NCH
                            for ci in range(nch_eff):
                                nc.tensor.matmul(
                                    ps[0:r, :],
                                    q_bf[ci][:, m * CP : m * CP + r],
                                    s_sc[ci][:, w * Y : (w + 1) * Y],
                                    start=(ci == 0),
                                    stop=(ci == nch_eff - 1),
                                )
                        m8a = m8a_tiles[m]
                        if ABLATE_MAX:
                            nc.vector.memset(m8a[0:r, w * 8 : (w + 1) * 8], 0.5)
                        elif MAX_FROM_PSUM:
                            nc.vector.max(m8a[0:r, w * 8 : (w + 1) * 8], ps[0:r, :])
                        else:
                            sco = sb_loop.tile([CP, Y], BF16, tag="sco")
                            nc.scalar.copy(sco[0:r, :], ps[0:r, :])
                            m8b = sb_loop.tile([CP, 8], BF16, tag="m8b")
                            nc.vector.max(m8b[0:r, :], sco[0:r, :])
                            nc.vector.tensor_copy(
                                m8a[0:r, w * 8 : (w + 1) * 8], m8b[0:r, :]
                            )
                        if last_way:
                            r2 = _rows(m)
                            v = sb_loop.tile([CP, WAY * KNN], F32, tag="v")
                            nc.scalar.activation(
                                v[0:r2, :],
                                m8a[0:r2, :].rearrange("p (w k) -> p w k", k=8)[
                                    :, :, 0:KNN
                                ],
                                AF.Square,
                                scale=qcol[0:r2, m : m + 1],
                            )
                            nc.tensor.matmul(
                                dn4ps[:],
                                seg_sb[0:r2, m * NQ : (m + 1) * NQ],
                                v[0:r2, :],
                                start=(m == 0),
                                stop=(m == NM - 1),
                                skip_group_check=True,
                            )
                nc.vector.tensor_reduce(
                    dn4r[:],
                    dn4ps[:].rearrange("p (w k) -> p w k", k=KNN),
                    AX.X,
                    AluOpType.add,
                )
                nc.scalar.activation(dn4_sb[:], dn4r[:], AF.Sqrt, scale=DN4_SCALE)
                nc.sync.dma_start(out_dn4[:], dn4_sb[:])


_NC_CACHE = None


def _get_program():
    global _NC_CACHE
    if _NC_CACHE is None:
        _NC_CACHE = build_program()
    return _NC_CACHE


def _make_seg():
    seg = np.zeros((CP, NM * NQ), np.float32)
    for m in range(NM):
        for p in range(_rows(m)):
            x = m * CP + p
            seg[p, m * NQ + x // HWDIM] = 1.0
    return seg


def _pack_q_fp8(xq_r):
    # xq_r: [NCH, CP, NQ*HWDIM] float32 -> [3, CP, NM*2*CP] fp8e4m3 with the
    # descriptor dim padded 2500->2560 and a zero 6th K-chunk.
    import ml_dtypes

    f8 = np.dtype(ml_dtypes.float8_e4m3)
    out = np.zeros((3, CP, NM, 2, CP), np.float32)
    padded = np.zeros((NCH, CP, NM * CP), np.float32)
    padded[:, :, : NQ * HWDIM] = xq_r
    for ci in range(NCH):
        out[ci // 2, :, :, ci % 2, :] = padded[ci].reshape(CP, NM, CP)
    return np.ascontiguousarray(out.astype(f8).reshape(3, CP, NM * 2 * CP))


def make_in_maps(x_shot, x_query):
    import ml_dtypes

    bf = np.dtype(ml_dtypes.bfloat16)
    x_shot = np.asarray(x_shot, dtype=np.float32).astype(bf)
    x_query = np.asarray(x_query, dtype=np.float32).astype(bf)
    seg = _make_seg()
    in_maps = []
    for core in range(8):
        b, h = divmod(core, 2)
        xs_r = (
            x_shot[b]
            .reshape(WAY * SHOT, NCH, CP, HWDIM)
            .transpose(1, 2, 0, 3)
            .reshape(NCH, CP, WAY * Y)
        )
        xq_r = (
            x_query[b, h * NQ : (h + 1) * NQ]
            .reshape(NQ, NCH, CP, HWDIM)
            .transpose(1, 2, 0, 3)
            .reshape(NCH, CP, NQ * HWDIM)
        )
        m = {
            "xs": np.ascontiguousarray(xs_r),
            "xq": np.ascontiguousarray(xq_r),
            "seg": seg,
        }
        if USE_FP8:
            m["xqf8"] = _pack_q_fp8(xq_r.astype(np.float32))
        in_maps.append(m)
    return in_maps


def assemble(results):
    cos = np.zeros((B, 50, WAY), np.float32)
    dn4 = np.zeros((B, 50, WAY), np.float32)
    for core in range(8):
        b, h = divmod(core, 2)
        cos[b, h * NQ : (h + 1) * NQ] = results[core]["out_cos"]
        dn4[b, h * NQ : (h + 1) * NQ] = results[core]["out_dn4"].reshape(NQ, WAY)
    return cos, dn4


def kernel(x_shot, x_query, r_cos, r_dn4, neighbor_k=5):
    from concourse.bass_utils import run_bass_kernel_spmd

    nc = _get_program()
    in_maps = make_in_maps(x_shot, x_query)
    r = run_bass_kernel_spmd(nc, in_maps, list(range(8)))
    cos, dn4 = assemble(r.results)
    return (
        cos,
        dn4,
        np.asarray(r_cos, np.float32),
        np.asarray(r_dn4, np.float32),
    )
